# revision 7
# baseline (speedup 1.0000x reference)
"""AreaAttention Trainium2 kernel: 8-core data-parallel over batch.

Each core processes one [512, 64, 64] image through:
  qk = SiLU(BN(conv1x1(x)));  v = SiLU(BN(conv1x1(x)))
  pp = SiLU(BN(conv3x3(v)))
  area attention (4 windows of 1024 tokens, 8 heads of dim 64) over qk/v
  y = SiLU(BN(conv1x1(attn_out + pp)))

BN scales are folded into conv weights on the host; BN biases are applied
in the ScalarE SiLU eviction. All matmuls run in bf16 with fp32 PSUM.

conv3x3 runs on flat [C, 4096] feature maps using shifted contiguous
slices per tap. Horizontal padding is handled with two extra copies of v
(one with column 63 zeroed for the kx=0 taps, one with column 0 zeroed
for kx=2) so row-wrap reads hit zeros; vertical padding is handled by
clipping each matmul's range (PSUM simply doesn't accumulate there).

Attention per (window, head): S^T = K^T Q on PE, exp on ScalarE with the
1/8 scale folded in, then attn@V as V_aug^T @ exp(S^T) where V_aug has a
ones column appended -- its output row 64 is the softmax denominator.
Normalization multiplies by the broadcast reciprocal (DRAM-bounce
partition broadcast).
"""

import numpy as np

import concourse.bacc as bacc
import concourse.bass as bass
from concourse import mybir
from concourse.tile import TileContext
from concourse.masks import make_identity

P = 128
C = 512
CI = C // P          # 4 input-channel chunks
OCQK = 2 * C // P    # 8 qk output chunks
OC = C // P          # 4 output chunks
HW = 4096            # 64*64 tokens
NCH = HW // 512      # 8 n-chunks of 512
WIN = 4              # area windows
NW = HW // WIN       # 1024 tokens per window
HEADS = 8
HD = 64
JC = NW // P         # 8 key chunks per window
EPS = 1e-5
FP32 = mybir.dt.float32
BF16 = mybir.dt.bfloat16
SILU = mybir.ActivationFunctionType.Silu
EXP = mybir.ActivationFunctionType.Exp

# taps ordered center-first so the full-range matmul opens each PSUM group
TAPS = [(1, 1)] + [(ky, kx) for ky in range(3) for kx in range(3) if (ky, kx) != (1, 1)]


def _build():
    nc = bacc.Bacc(None, target_bir_lowering=False, debug=False)

    x_ext = nc.declare_dram_parameter("x", [C, HW], FP32, isOutput=False)
    qkw_ext = nc.declare_dram_parameter("qk_wt", [C, 2 * C], FP32, isOutput=False)
    vw_ext = nc.declare_dram_parameter("v_wt", [C, C], FP32, isOutput=False)
    pew_ext = nc.declare_dram_parameter("pe_wt", [9, C, C], FP32, isOutput=False)
    prw_ext = nc.declare_dram_parameter("pr_wt", [C, C], FP32, isOutput=False)
    bqk_ext = nc.declare_dram_parameter("b_qk", [2 * C], FP32, isOutput=False)
    bv_ext = nc.declare_dram_parameter("b_v", [C], FP32, isOutput=False)
    bpe_ext = nc.declare_dram_parameter("b_pe", [C], FP32, isOutput=False)
    bpr_ext = nc.declare_dram_parameter("b_pr", [C], FP32, isOutput=False)
    out_ext = nc.declare_dram_parameter("out", [C, HW], FP32, isOutput=True)

    with TileContext(nc) as tc:
        with (
            tc.tile_pool(name="const", bufs=1) as const_pool,
            tc.tile_pool(name="persist", bufs=1) as persist,
            tc.tile_pool(name="dram", bufs=1, space="DRAM") as dram,
            tc.tile_pool(name="dram2", bufs=3, space="DRAM") as dram2,
            tc.tile_pool(name="psum_mm", bufs=2, space="PSUM") as psum_mm,
        ):
            ident = const_pool.tile([P, P], BF16)
            make_identity(nc, ident)

            # biases, rearranged so partition = channel % 128
            bqk_sb = const_pool.tile([P, OCQK], FP32)
            nc.sync.dma_start(
                out=bqk_sb[:], in_=bqk_ext[:].rearrange("(oc p) -> p oc", p=P)
            )
            bv_sb = const_pool.tile([P, OC], FP32)
            nc.sync.dma_start(
                out=bv_sb[:], in_=bv_ext[:].rearrange("(oc p) -> p oc", p=P)
            )
            bpe_sb = const_pool.tile([P, OC], FP32)
            nc.sync.dma_start(
                out=bpe_sb[:], in_=bpe_ext[:].rearrange("(oc p) -> p oc", p=P)
            )
            bpr_sb = const_pool.tile([P, OC], FP32)
            nc.sync.dma_start(
                out=bpr_sb[:], in_=bpr_ext[:].rearrange("(oc p) -> p oc", p=P)
            )

            # small weights, bf16 [p, ci, o]
            qk_bf = persist.tile([P, CI, 2 * C], BF16)
            v_bf = persist.tile([P, CI, C], BF16)
            pr_bf = persist.tile([P, CI, C], BF16)

            # v feature map (flat) -- attention V source and conv3x3 center
            v_mid = persist.tile([P, CI, HW], BF16)

            qk_dram = dram.tile([2 * C, HW], BF16)
            attn_dram = dram.tile([P, CI, HW], BF16)
            pp_dram = dram.tile([P, CI, HW], BF16)

            with tc.tile_pool(name="vlr", bufs=1) as vlr:
                v_l = vlr.tile([P, CI, HW], BF16)   # col 63 zeroed (kx=0 taps)
                v_r = vlr.tile([P, CI, HW], BF16)   # col 0 zeroed (kx=2 taps)
                vtap = {0: v_l, 1: v_mid, 2: v_r}

                # ------------ Phase A: load x + weights, qk/v convs ----
                with (
                    tc.tile_pool(name="xpool", bufs=1) as xpool,
                    tc.tile_pool(name="ldpool", bufs=2) as ldpool,
                    tc.tile_pool(name="stpool", bufs=3) as stpool,
                ):
                    wld = ldpool.tile([P, CI, 2 * C], FP32, tag="ld")
                    nc.sync.dma_start(
                        out=wld[:],
                        in_=qkw_ext[:].rearrange("(cc p) o -> p cc o", p=P),
                    )
                    nc.vector.tensor_copy(qk_bf[:], wld[:])
                    wld2 = ldpool.tile([P, CI, C], FP32, tag="ld")
                    nc.sync.dma_start(
                        out=wld2[:],
                        in_=vw_ext[:].rearrange("(cc p) o -> p cc o", p=P),
                    )
                    nc.vector.tensor_copy(v_bf[:], wld2[:])
                    wld3 = ldpool.tile([P, CI, C], FP32, tag="ld")
                    nc.sync.dma_start(
                        out=wld3[:],
                        in_=prw_ext[:].rearrange("(cc p) o -> p cc o", p=P),
                    )
                    nc.vector.tensor_copy(pr_bf[:], wld3[:])

                    x_bf = xpool.tile([P, CI, HW], BF16)
                    for ci in range(CI):
                        xld = ldpool.tile([P, HW], FP32, tag="ld")
                        nc.sync.dma_start(
                            out=xld[:], in_=x_ext[ci * P : (ci + 1) * P, :]
                        )
                        nc.vector.tensor_copy(x_bf[:, ci, :], xld[:])

                    # qk conv1x1 -> silu -> qk_dram
                    for oc in range(OCQK):
                        for nch in range(NCH):
                            ps = psum_mm.tile([P, 512], FP32, tag="mm")
                            for ci in range(CI):
                                nc.tensor.matmul(
                                    ps[:],
                                    qk_bf[:, ci, oc * P : (oc + 1) * P],
                                    x_bf[:, ci, nch * 512 : (nch + 1) * 512],
                                    start=(ci == 0),
                                    stop=(ci == CI - 1),
                                )
                            st = stpool.tile([P, 512], BF16, tag="st")
                            nc.scalar.activation(
                                st[:], ps[:], SILU, bias=bqk_sb[:, oc : oc + 1]
                            )
                            nc.sync.dma_start(
                                out=qk_dram[
                                    oc * P : (oc + 1) * P,
                                    nch * 512 : (nch + 1) * 512,
                                ],
                                in_=st[:],
                            )

                    # v conv1x1 -> silu -> v_mid
                    for oc in range(OC):
                        for nch in range(NCH):
                            ps = psum_mm.tile([P, 512], FP32, tag="mm")
                            for ci in range(CI):
                                nc.tensor.matmul(
                                    ps[:],
                                    v_bf[:, ci, oc * P : (oc + 1) * P],
                                    x_bf[:, ci, nch * 512 : (nch + 1) * 512],
                                    start=(ci == 0),
                                    stop=(ci == CI - 1),
                                )
                            nc.scalar.activation(
                                v_mid[:, oc, nch * 512 : (nch + 1) * 512],
                                ps[:],
                                SILU,
                                bias=bv_sb[:, oc : oc + 1],
                            )

                    # boundary copies for conv3x3 horizontal padding
                    nc.vector.tensor_copy(v_l[:], v_mid[:])
                    nc.vector.tensor_copy(v_r[:], v_mid[:])
                    nc.vector.memset(
                        v_l[:].rearrange("p c (r w) -> p c r w", w=64)[:, :, :, 63:64],
                        0,
                    )
                    nc.vector.memset(
                        v_r[:].rearrange("p c (r w) -> p c r w", w=64)[:, :, :, 0:1],
                        0,
                    )

                # ------------ Phase B: conv3x3 -> pp_dram ---------------
                with (
                    tc.tile_pool(name="pepool", bufs=1) as pepool,
                    tc.tile_pool(name="peld", bufs=3) as peld,
                    tc.tile_pool(name="stpool2", bufs=3) as stpool2,
                ):
                    pe_bf = pepool.tile([P, CI, 9, C], BF16)
                    for ci in range(CI):
                        for tap in range(9):
                            pld = peld.tile([P, C], FP32, tag="peld")
                            nc.sync.dma_start(
                                out=pld[:],
                                in_=pew_ext[tap, ci * P : (ci + 1) * P, :],
                            )
                            nc.vector.tensor_copy(pe_bf[:, ci, tap, :], pld[:])

                    for oc in range(OC):
                        for nch in range(NCH):
                            n0 = nch * 512
                            ps = psum_mm.tile([P, 512], FP32, tag="mm")
                            mms = []
                            for ky, kx in TAPS:
                                s = (ky - 1) * 64 + (kx - 1)
                                lo = max(0, -s - n0)
                                hi = min(512, HW - s - n0)
                                src = vtap[kx]
                                for ci in range(CI):
                                    mms.append((
                                        ps[:, lo:hi],
                                        pe_bf[:, ci, ky * 3 + kx, oc * P : (oc + 1) * P],
                                        src[:, ci, n0 + s + lo : n0 + s + hi],
                                    ))
                            for i, (o, l, r) in enumerate(mms):
                                nc.tensor.matmul(
                                    o, l, r,
                                    start=(i == 0),
                                    stop=(i == len(mms) - 1),
                                    skip_group_check=True,
                                )
                            st = stpool2.tile([P, 512], BF16, tag="st2")
                            nc.scalar.activation(
                                st[:], ps[:], SILU, bias=bpe_sb[:, oc : oc + 1]
                            )
                            nc.sync.dma_start(
                                out=pp_dram[:, oc, n0 : n0 + 512], in_=st[:]
                            )

            # ---------------- Phase C: area attention ------------------
            with (
                tc.tile_pool(name="vaugp", bufs=2) as vaugp,
                tc.tile_pool(name="aexpp", bufs=2) as aexpp,
                tc.tile_pool(name="qkp", bufs=2) as qkp,
                tc.tile_pool(name="recipp", bufs=2) as recipp,
                tc.tile_pool(name="tmpp", bufs=2) as tmpp,
                tc.tile_pool(name="psA", bufs=2, space="PSUM") as psA,
                tc.tile_pool(name="psO", bufs=1, space="PSUM") as psO,
            ):
                for w in range(WIN):
                    # transpose V for this window: vaug[j, jc, h, 0:64]=v^T
                    vaug = vaugp.tile([P, JC, HEADS, HD + 1], BF16, tag="vaug")
                    nc.vector.memset(vaug[:, :, :, HD : HD + 1], 1.0)
                    for jc in range(JC):
                        t0 = w * NW + jc * P
                        for ci in range(CI):
                            pt = psum_mm.tile([P, P], BF16, tag="mm")
                            nc.tensor.transpose(
                                pt[:], v_mid[:, ci, t0 : t0 + P], ident[:]
                            )
                            nc.vector.tensor_copy(
                                vaug[:, jc, 2 * ci : 2 * ci + 2, 0:HD],
                                pt[:].rearrange("p (a b) -> p a b", a=2),
                            )

                    for h in range(HEADS):
                        q_t = qkp.tile([HD, NW], BF16, tag="q")
                        nc.sync.dma_start(
                            out=q_t[:],
                            in_=qk_dram[
                                h * HD : (h + 1) * HD, w * NW : (w + 1) * NW
                            ],
                        )
                        k_t = qkp.tile([HD, NW], BF16, tag="k")
                        nc.sync.dma_start(
                            out=k_t[:],
                            in_=qk_dram[
                                C + h * HD : C + (h + 1) * HD, w * NW : (w + 1) * NW
                            ],
                        )

                        a_exp = aexpp.tile([P, JC, NW], BF16, tag="aexp")
                        for jc in range(JC):
                            ps_s = psA.tile([P, NW], FP32, tag="s")
                            for half in range(2):
                                nc.tensor.matmul(
                                    ps_s[:, half * 512 : (half + 1) * 512],
                                    k_t[:, jc * P : (jc + 1) * P],
                                    q_t[:, half * 512 : (half + 1) * 512],
                                    start=True,
                                    stop=True,
                                )
                            nc.scalar.activation(
                                a_exp[:, jc, :], ps_s[:], EXP, scale=0.125
                            )

                        ps_o = psO.tile([HD + 1, NW], FP32, tag="o")
                        for half in range(2):
                            for jc in range(JC):
                                nc.tensor.matmul(
                                    ps_o[:, half * 512 : (half + 1) * 512],
                                    vaug[:, jc, h, :],
                                    a_exp[:, jc, half * 512 : (half + 1) * 512],
                                    start=(jc == 0),
                                    stop=(jc == JC - 1),
                                )

                        # softmax denominators live in row 64; divide rows 0..63
                        srow = recipp.tile([HD + 1, NW], FP32, tag="srow")
                        nc.vector.reciprocal(
                            srow[HD : HD + 1, :], ps_o[HD : HD + 1, :]
                        )
                        # broadcast across partitions via a DRAM bounce
                        rrow = dram2.tile([1, NW], FP32, tag="rrow")
                        nc.sync.dma_start(out=rrow[:], in_=srow[HD : HD + 1, :])
                        rbc = recipp.tile([HD, NW], FP32, tag="rbc")
                        nc.sync.dma_start(
                            out=rbc[:], in_=rrow[:].partition_broadcast(HD)
                        )
                        tmp = tmpp.tile([HD, NW], BF16, tag="tmp")
                        nc.vector.tensor_mul(tmp[:], ps_o[0:HD, :], rbc[:])
                        nc.sync.dma_start(
                            out=attn_dram[
                                (h % 2) * HD : (h % 2) * HD + HD,
                                h // 2,
                                w * NW : (w + 1) * NW,
                            ],
                            in_=tmp[:],
                        )

            # ---------------- Phase D: pr conv1x1 ----------------------
            with (
                tc.tile_pool(name="aldp", bufs=2) as aldp,
                tc.tile_pool(name="zp", bufs=2) as zp,
                tc.tile_pool(name="ystp", bufs=3) as ystp,
            ):
                for nch in range(NCH):
                    a_ld = aldp.tile([P, CI, 512], BF16, tag="ald")
                    nc.sync.dma_start(
                        out=a_ld[:],
                        in_=attn_dram[:, :, nch * 512 : (nch + 1) * 512],
                    )
                    p_ld = aldp.tile([P, CI, 512], BF16, tag="pld")
                    nc.sync.dma_start(
                        out=p_ld[:],
                        in_=pp_dram[:, :, nch * 512 : (nch + 1) * 512],
                    )
                    z = zp.tile([P, CI, 512], BF16, tag="z")
                    nc.vector.tensor_add(z[:], p_ld[:], a_ld[:])
                    for oc in range(OC):
                        ps = psum_mm.tile([P, 512], FP32, tag="mm")
                        for ci in range(CI):
                            nc.tensor.matmul(
                                ps[:],
                                pr_bf[:, ci, oc * P : (oc + 1) * P],
                                z[:, ci, :],
                                start=(ci == 0),
                                stop=(ci == CI - 1),
                            )
                        yst = ystp.tile([P, 512], FP32, tag="yst")
                        nc.scalar.activation(
                            yst[:], ps[:], SILU, bias=bpr_sb[:, oc : oc + 1]
                        )
                        nc.sync.dma_start(
                            out=out_ext[
                                oc * P : (oc + 1) * P, nch * 512 : (nch + 1) * 512
                            ],
                            in_=yst[:],
                        )

    nc.compile()
    return nc


_NC_CACHE = {}


def _get_nc():
    if "nc" not in _NC_CACHE:
        _NC_CACHE["nc"] = _build()
    return _NC_CACHE["nc"]


def _make_in_maps(inputs):
    x = np.asarray(inputs["x"], dtype=np.float32)          # [8, 512, 64, 64]
    B = x.shape[0]

    def fold(wname, gname, bname, mname, vname):
        g = np.asarray(inputs[gname], np.float32)
        b = np.asarray(inputs[bname], np.float32)
        m = np.asarray(inputs[mname], np.float32)
        v = np.asarray(inputs[vname], np.float32)
        s = g / np.sqrt(v + EPS)
        w = np.asarray(inputs[wname], np.float32)
        return s, (b - m * s).astype(np.float32), w

    s_qk, b_qk, qk_w = fold("qk_w", "qk_g", "qk_b", "qk_rm", "qk_rv")
    s_v, b_v, v_w = fold("v_w", "v_g", "v_b", "v_rm", "v_rv")
    s_pe, b_pe, pe_w = fold("pe_w", "pe_g", "pe_b", "pe_rm", "pe_rv")
    s_pr, b_pr, pr_w = fold("pr_w", "pr_g", "pr_b", "pr_rm", "pr_rv")

    qk_wt = np.ascontiguousarray((qk_w * s_qk[:, None]).T)          # [512, 1024]
    v_wt = np.ascontiguousarray((v_w * s_v[:, None]).T)             # [512, 512]
    pr_wt = np.ascontiguousarray((pr_w * s_pr[:, None]).T)          # [512, 512]
    pe_wt = np.ascontiguousarray(
        (pe_w * s_pe[:, None, None, None]).transpose(2, 3, 1, 0).reshape(9, C, C)
    )                                                               # [9, 512, 512]

    shared = {
        "qk_wt": qk_wt, "v_wt": v_wt, "pe_wt": pe_wt, "pr_wt": pr_wt,
        "b_qk": b_qk, "b_v": b_v, "b_pe": b_pe, "b_pr": b_pr,
    }
    return [
        {"x": np.ascontiguousarray(x[i].reshape(C, HW)), **shared}
        for i in range(B)
    ]


def kernel(**inputs):
    from concourse.bass_utils import run_bass_kernel_spmd

    in_maps = _make_in_maps(inputs)
    B = len(in_maps)
    nc = _get_nc()
    res = run_bass_kernel_spmd(nc, in_maps, core_ids=list(range(B)))
    out = np.stack([res.results[i]["out"] for i in range(B)], axis=0)
    return out.reshape(B, C, 64, 64).astype(np.float32)


# revision 9
# speedup vs baseline: 1.2360x; 1.2360x over previous
"""AreaAttention Trainium2 kernel: 8-core data-parallel over batch.

Each core processes one [512, 64, 64] image through:
  qk = SiLU(BN(conv1x1(x)));  v = SiLU(BN(conv1x1(x)))
  pp = SiLU(BN(conv3x3(v)))
  area attention (4 windows of 1024 tokens, 8 heads of dim 64) over qk/v
  y = SiLU(BN(conv1x1(attn_out + pp)))

BN scales are folded into conv weights on the host; BN biases are applied
in the ScalarE SiLU eviction. All matmuls run in bf16 with fp32 PSUM.

conv3x3 runs on flat [C, 4096] feature maps using shifted contiguous
slices per tap. Horizontal padding is handled with two extra copies of v
(one with column 63 zeroed for the kx=0 taps, one with column 0 zeroed
for kx=2) so row-wrap reads hit zeros; vertical padding is handled by
clipping each matmul's range (PSUM simply doesn't accumulate there).

Attention per (window, head): S^T = K^T Q on PE, exp on ScalarE with the
1/8 scale folded in, then attn@V as V_aug^T @ exp(S^T) where V_aug has a
ones column appended -- its output row 64 is the softmax denominator.
Normalization multiplies by the broadcast reciprocal (DRAM-bounce
partition broadcast).
"""

import numpy as np

import concourse.bacc as bacc
import concourse.bass as bass
from concourse import mybir
from concourse.tile import TileContext
from concourse.masks import make_identity

P = 128
C = 512
CI = C // P          # 4 input-channel chunks
OCQK = 2 * C // P    # 8 qk output chunks
OC = C // P          # 4 output chunks
HW = 4096            # 64*64 tokens
NCH = HW // 512      # 8 n-chunks of 512
WIN = 4              # area windows
NW = HW // WIN       # 1024 tokens per window
HEADS = 8
HD = 64
JC = NW // P         # 8 key chunks per window
EPS = 1e-5
FP32 = mybir.dt.float32
BF16 = mybir.dt.bfloat16
SILU = mybir.ActivationFunctionType.Silu
EXP = mybir.ActivationFunctionType.Exp

# taps ordered center-first so the full-range matmul opens each PSUM group
TAPS = [(1, 1)] + [(ky, kx) for ky in range(3) for kx in range(3) if (ky, kx) != (1, 1)]


def _build():
    nc = bacc.Bacc(None, target_bir_lowering=False, debug=False)

    x_ext = nc.declare_dram_parameter("x", [C, HW], FP32, isOutput=False)
    qkw_ext = nc.declare_dram_parameter("qk_wt", [C, 2 * C], FP32, isOutput=False)
    vw_ext = nc.declare_dram_parameter("v_wt", [C, C], FP32, isOutput=False)
    pew_ext = nc.declare_dram_parameter("pe_wt", [9, C, C], FP32, isOutput=False)
    prw_ext = nc.declare_dram_parameter("pr_wt", [C, C], FP32, isOutput=False)
    bqk_ext = nc.declare_dram_parameter("b_qk", [2 * C], FP32, isOutput=False)
    bv_ext = nc.declare_dram_parameter("b_v", [C], FP32, isOutput=False)
    bpe_ext = nc.declare_dram_parameter("b_pe", [C], FP32, isOutput=False)
    bpr_ext = nc.declare_dram_parameter("b_pr", [C], FP32, isOutput=False)
    out_ext = nc.declare_dram_parameter("out", [C, HW], FP32, isOutput=True)

    with TileContext(nc) as tc:
        with (
            tc.tile_pool(name="const", bufs=1) as const_pool,
            tc.tile_pool(name="persist", bufs=1) as persist,
            tc.tile_pool(name="dram", bufs=1, space="DRAM") as dram,
            tc.tile_pool(name="dram2", bufs=3, space="DRAM") as dram2,
        ):
            ident = const_pool.tile([P, P], BF16)
            make_identity(nc, ident)

            # biases, rearranged so partition = channel % 128
            bqk_sb = const_pool.tile([P, OCQK], FP32)
            nc.sync.dma_start(
                out=bqk_sb[:], in_=bqk_ext[:].rearrange("(oc p) -> p oc", p=P)
            )
            bv_sb = const_pool.tile([P, OC], FP32)
            nc.sync.dma_start(
                out=bv_sb[:], in_=bv_ext[:].rearrange("(oc p) -> p oc", p=P)
            )
            bpe_sb = const_pool.tile([P, OC], FP32)
            nc.sync.dma_start(
                out=bpe_sb[:], in_=bpe_ext[:].rearrange("(oc p) -> p oc", p=P)
            )
            bpr_sb = const_pool.tile([P, OC], FP32)
            nc.sync.dma_start(
                out=bpr_sb[:], in_=bpr_ext[:].rearrange("(oc p) -> p oc", p=P)
            )

            # small weights, bf16 [p, ci, o]
            qk_bf = persist.tile([P, CI, 2 * C], BF16)
            v_bf = persist.tile([P, CI, C], BF16)
            pr_bf = persist.tile([P, CI, C], BF16)

            # v feature map (flat) -- attention V source and conv3x3 center
            v_mid = persist.tile([P, CI, HW], BF16)

            qk_dram = dram.tile([2 * C, HW], BF16)
            attn_dram = dram.tile([P, CI, HW], BF16)
            pp_dram = dram.tile([P, CI, HW], BF16)

            with (
                tc.tile_pool(name="vlr", bufs=1) as vlr,
                tc.tile_pool(name="psum_ab", bufs=4, space="PSUM") as psum_mm,
            ):
                v_l = vlr.tile([P, CI, HW], BF16)   # col 63 zeroed (kx=0 taps)
                v_r = vlr.tile([P, CI, HW], BF16)   # col 0 zeroed (kx=2 taps)
                vtap = {0: v_l, 1: v_mid, 2: v_r}

                # ------------ Phase A: load x + weights, qk/v convs ----
                with (
                    tc.tile_pool(name="xpool", bufs=1) as xpool,
                    tc.tile_pool(name="ldpool", bufs=2) as ldpool,
                    tc.tile_pool(name="stpool", bufs=3) as stpool,
                ):
                    wld = ldpool.tile([P, CI, 2 * C], FP32, tag="ld")
                    nc.sync.dma_start(
                        out=wld[:],
                        in_=qkw_ext[:].rearrange("(cc p) o -> p cc o", p=P),
                    )
                    nc.vector.tensor_copy(qk_bf[:], wld[:])
                    wld2 = ldpool.tile([P, CI, C], FP32, tag="ld")
                    nc.sync.dma_start(
                        out=wld2[:],
                        in_=vw_ext[:].rearrange("(cc p) o -> p cc o", p=P),
                    )
                    nc.vector.tensor_copy(v_bf[:], wld2[:])
                    wld3 = ldpool.tile([P, CI, C], FP32, tag="ld")
                    nc.sync.dma_start(
                        out=wld3[:],
                        in_=prw_ext[:].rearrange("(cc p) o -> p cc o", p=P),
                    )
                    nc.vector.tensor_copy(pr_bf[:], wld3[:])

                    x_bf = xpool.tile([P, CI, HW], BF16)
                    for ci in range(CI):
                        xld = ldpool.tile([P, HW], FP32, tag="ld")
                        nc.sync.dma_start(
                            out=xld[:], in_=x_ext[ci * P : (ci + 1) * P, :]
                        )
                        nc.vector.tensor_copy(x_bf[:, ci, :], xld[:])

                    # v conv1x1 -> silu -> v_mid (+ boundary copies)
                    for oc in range(OC):
                        for nch in range(NCH):
                            ps = psum_mm.tile([P, 512], FP32, tag="mm")
                            for ci in range(CI):
                                nc.tensor.matmul(
                                    ps[:],
                                    v_bf[:, ci, oc * P : (oc + 1) * P],
                                    x_bf[:, ci, nch * 512 : (nch + 1) * 512],
                                    start=(ci == 0),
                                    stop=(ci == CI - 1),
                                )
                            sl = v_mid[:, oc, nch * 512 : (nch + 1) * 512]
                            nc.scalar.activation(
                                sl, ps[:], SILU, bias=bv_sb[:, oc : oc + 1]
                            )
                            nc.vector.tensor_copy(
                                v_l[:, oc, nch * 512 : (nch + 1) * 512], sl
                            )
                            nc.vector.tensor_copy(
                                v_r[:, oc, nch * 512 : (nch + 1) * 512], sl
                            )
                    nc.vector.memset(
                        v_l[:].rearrange("p c (r w) -> p c r w", w=64)[:, :, :, 63:64],
                        0,
                    )
                    nc.vector.memset(
                        v_r[:].rearrange("p c (r w) -> p c r w", w=64)[:, :, :, 0:1],
                        0,
                    )

                    # qk conv1x1 -> silu -> qk_dram
                    for oc in range(OCQK):
                        for nch in range(NCH):
                            ps = psum_mm.tile([P, 512], FP32, tag="mm")
                            for ci in range(CI):
                                nc.tensor.matmul(
                                    ps[:],
                                    qk_bf[:, ci, oc * P : (oc + 1) * P],
                                    x_bf[:, ci, nch * 512 : (nch + 1) * 512],
                                    start=(ci == 0),
                                    stop=(ci == CI - 1),
                                )
                            st = stpool.tile([P, 512], BF16, tag="st")
                            nc.scalar.activation(
                                st[:], ps[:], SILU, bias=bqk_sb[:, oc : oc + 1]
                            )
                            nc.sync.dma_start(
                                out=qk_dram[
                                    oc * P : (oc + 1) * P,
                                    nch * 512 : (nch + 1) * 512,
                                ],
                                in_=st[:],
                            )

                # ------------ Phase B: conv3x3 -> pp_dram ---------------
                with (
                    tc.tile_pool(name="pepool", bufs=1) as pepool,
                    tc.tile_pool(name="peld", bufs=3) as peld,
                    tc.tile_pool(name="stpool2", bufs=3) as stpool2,
                ):
                    pe_bf = pepool.tile([P, CI, 9, C], BF16)
                    for ci in range(CI):
                        for tap in range(9):
                            pld = peld.tile([P, C], FP32, tag="peld")
                            nc.sync.dma_start(
                                out=pld[:],
                                in_=pew_ext[tap, ci * P : (ci + 1) * P, :],
                            )
                            nc.vector.tensor_copy(pe_bf[:, ci, tap, :], pld[:])

                    for oc in range(OC):
                        for nch in range(NCH):
                            n0 = nch * 512
                            ps = psum_mm.tile([P, 512], FP32, tag="mm")
                            mms = []
                            for ky, kx in TAPS:
                                s = (ky - 1) * 64 + (kx - 1)
                                lo = max(0, -s - n0)
                                hi = min(512, HW - s - n0)
                                src = vtap[kx]
                                for ci in range(CI):
                                    mms.append((
                                        ps[:, lo:hi],
                                        pe_bf[:, ci, ky * 3 + kx, oc * P : (oc + 1) * P],
                                        src[:, ci, n0 + s + lo : n0 + s + hi],
                                    ))
                            for i, (o, l, r) in enumerate(mms):
                                nc.tensor.matmul(
                                    o, l, r,
                                    start=(i == 0),
                                    stop=(i == len(mms) - 1),
                                    skip_group_check=True,
                                )
                            st = stpool2.tile([P, 512], BF16, tag="st2")
                            nc.scalar.activation(
                                st[:], ps[:], SILU, bias=bpe_sb[:, oc : oc + 1]
                            )
                            nc.sync.dma_start(
                                out=pp_dram[:, oc, n0 : n0 + 512], in_=st[:]
                            )

            # ---------------- Phase C: area attention ------------------
            with (
                tc.tile_pool(name="vaugp", bufs=2) as vaugp,
                tc.tile_pool(name="aexpp", bufs=2) as aexpp,
                tc.tile_pool(name="qkp", bufs=2) as qkp,
                tc.tile_pool(name="recipp", bufs=2) as recipp,
                tc.tile_pool(name="tmpp", bufs=2) as tmpp,
                tc.tile_pool(name="psA", bufs=2, space="PSUM") as psA,
                tc.tile_pool(name="psO", bufs=2, space="PSUM") as psO,
            ):
                for w in range(WIN):
                    # transpose V for this window: vaug[j, jc, h, 0:64]=v^T
                    vaug = vaugp.tile([P, JC, HEADS, HD + 1], BF16, tag="vaug")
                    nc.vector.memset(vaug[:, :, :, HD : HD + 1], 1.0)
                    for jc in range(JC):
                        t0 = w * NW + jc * P
                        for ci in range(CI):
                            pt = psA.tile([P, P], BF16, tag="s")
                            nc.tensor.transpose(
                                pt[:], v_mid[:, ci, t0 : t0 + P], ident[:]
                            )
                            nc.vector.tensor_copy(
                                vaug[:, jc, 2 * ci : 2 * ci + 2, 0:HD],
                                pt[:].rearrange("p (a b) -> p a b", a=2),
                            )

                    for h in range(HEADS):
                        q_t = qkp.tile([HD, NW], BF16, tag="q")
                        nc.sync.dma_start(
                            out=q_t[:],
                            in_=qk_dram[
                                h * HD : (h + 1) * HD, w * NW : (w + 1) * NW
                            ],
                        )
                        k_t = qkp.tile([HD, NW], BF16, tag="k")
                        nc.sync.dma_start(
                            out=k_t[:],
                            in_=qk_dram[
                                C + h * HD : C + (h + 1) * HD, w * NW : (w + 1) * NW
                            ],
                        )

                        a_exp = aexpp.tile([P, JC, NW], BF16, tag="aexp")
                        for jc in range(JC):
                            ps_s = psA.tile([P, NW], FP32, tag="s")
                            for half in range(2):
                                nc.tensor.matmul(
                                    ps_s[:, half * 512 : (half + 1) * 512],
                                    k_t[:, jc * P : (jc + 1) * P],
                                    q_t[:, half * 512 : (half + 1) * 512],
                                    start=True,
                                    stop=True,
                                )
                            nc.scalar.activation(
                                a_exp[:, jc, :], ps_s[:], EXP, scale=0.125
                            )

                        ps_o = psO.tile([HD + 1, NW], FP32, tag="o")
                        for half in range(2):
                            for jc in range(JC):
                                nc.tensor.matmul(
                                    ps_o[:, half * 512 : (half + 1) * 512],
                                    vaug[:, jc, h, :],
                                    a_exp[:, jc, half * 512 : (half + 1) * 512],
                                    start=(jc == 0),
                                    stop=(jc == JC - 1),
                                )

                        # softmax denominators live in row 64; reciprocal is
                        # per-column-cost on DVE, so reshape 1x1024 -> 128x8
                        # via DRAM, invert there, and broadcast back.
                        srow = recipp.tile([HD + 1, NW], FP32, tag="srow")
                        nc.vector.tensor_copy(
                            srow[HD : HD + 1, :], ps_o[HD : HD + 1, :]
                        )
                        rrow = dram2.tile([NW], FP32, tag="rrow")
                        nc.sync.dma_start(out=rrow[:], in_=srow[HD : HD + 1, :])
                        r128 = recipp.tile([P, JC], FP32, tag="r128")
                        nc.sync.dma_start(
                            out=r128[:], in_=rrow[:].rearrange("(p c) -> p c", p=P)
                        )
                        nc.vector.reciprocal(r128[:], r128[:])
                        rrec = dram2.tile([NW], FP32, tag="rrec")
                        nc.sync.dma_start(
                            out=rrec[:].rearrange("(p c) -> p c", p=P), in_=r128[:]
                        )
                        rbc = recipp.tile([HD, NW], FP32, tag="rbc")
                        nc.sync.dma_start(
                            out=rbc[:],
                            in_=rrec[:].unsqueeze(0).partition_broadcast(HD),
                        )
                        tmp = tmpp.tile([HD, NW], BF16, tag="tmp")
                        nc.vector.tensor_mul(tmp[:], ps_o[0:HD, :], rbc[:])
                        nc.sync.dma_start(
                            out=attn_dram[
                                (h % 2) * HD : (h % 2) * HD + HD,
                                h // 2,
                                w * NW : (w + 1) * NW,
                            ],
                            in_=tmp[:],
                        )

            # ---------------- Phase D: pr conv1x1 ----------------------
            with (
                tc.tile_pool(name="psum_d", bufs=4, space="PSUM") as psum_mm,
                tc.tile_pool(name="aldp", bufs=2) as aldp,
                tc.tile_pool(name="zp", bufs=2) as zp,
                tc.tile_pool(name="ystp", bufs=3) as ystp,
            ):
                for nch in range(NCH):
                    a_ld = aldp.tile([P, CI, 512], BF16, tag="ald")
                    nc.sync.dma_start(
                        out=a_ld[:],
                        in_=attn_dram[:, :, nch * 512 : (nch + 1) * 512],
                    )
                    p_ld = aldp.tile([P, CI, 512], BF16, tag="pld")
                    nc.sync.dma_start(
                        out=p_ld[:],
                        in_=pp_dram[:, :, nch * 512 : (nch + 1) * 512],
                    )
                    z = zp.tile([P, CI, 512], BF16, tag="z")
                    nc.vector.tensor_add(z[:], p_ld[:], a_ld[:])
                    for oc in range(OC):
                        ps = psum_mm.tile([P, 512], FP32, tag="mm")
                        for ci in range(CI):
                            nc.tensor.matmul(
                                ps[:],
                                pr_bf[:, ci, oc * P : (oc + 1) * P],
                                z[:, ci, :],
                                start=(ci == 0),
                                stop=(ci == CI - 1),
                            )
                        yst = ystp.tile([P, 512], FP32, tag="yst")
                        nc.scalar.activation(
                            yst[:], ps[:], SILU, bias=bpr_sb[:, oc : oc + 1]
                        )
                        nc.sync.dma_start(
                            out=out_ext[
                                oc * P : (oc + 1) * P, nch * 512 : (nch + 1) * 512
                            ],
                            in_=yst[:],
                        )

    nc.compile()
    return nc


_NC_CACHE = {}


def _get_nc():
    if "nc" not in _NC_CACHE:
        _NC_CACHE["nc"] = _build()
    return _NC_CACHE["nc"]


def _make_in_maps(inputs):
    x = np.asarray(inputs["x"], dtype=np.float32)          # [8, 512, 64, 64]
    B = x.shape[0]

    def fold(wname, gname, bname, mname, vname):
        g = np.asarray(inputs[gname], np.float32)
        b = np.asarray(inputs[bname], np.float32)
        m = np.asarray(inputs[mname], np.float32)
        v = np.asarray(inputs[vname], np.float32)
        s = g / np.sqrt(v + EPS)
        w = np.asarray(inputs[wname], np.float32)
        return s, (b - m * s).astype(np.float32), w

    s_qk, b_qk, qk_w = fold("qk_w", "qk_g", "qk_b", "qk_rm", "qk_rv")
    s_v, b_v, v_w = fold("v_w", "v_g", "v_b", "v_rm", "v_rv")
    s_pe, b_pe, pe_w = fold("pe_w", "pe_g", "pe_b", "pe_rm", "pe_rv")
    s_pr, b_pr, pr_w = fold("pr_w", "pr_g", "pr_b", "pr_rm", "pr_rv")

    qk_wt = np.ascontiguousarray((qk_w * s_qk[:, None]).T)          # [512, 1024]
    v_wt = np.ascontiguousarray((v_w * s_v[:, None]).T)             # [512, 512]
    pr_wt = np.ascontiguousarray((pr_w * s_pr[:, None]).T)          # [512, 512]
    pe_wt = np.ascontiguousarray(
        (pe_w * s_pe[:, None, None, None]).transpose(2, 3, 1, 0).reshape(9, C, C)
    )                                                               # [9, 512, 512]

    shared = {
        "qk_wt": qk_wt, "v_wt": v_wt, "pe_wt": pe_wt, "pr_wt": pr_wt,
        "b_qk": b_qk, "b_v": b_v, "b_pe": b_pe, "b_pr": b_pr,
    }
    return [
        {"x": np.ascontiguousarray(x[i].reshape(C, HW)), **shared}
        for i in range(B)
    ]


def kernel(**inputs):
    from concourse.bass_utils import run_bass_kernel_spmd

    in_maps = _make_in_maps(inputs)
    B = len(in_maps)
    nc = _get_nc()
    res = run_bass_kernel_spmd(nc, in_maps, core_ids=list(range(B)))
    out = np.stack([res.results[i]["out"] for i in range(B)], axis=0)
    return out.reshape(B, C, 64, 64).astype(np.float32)
